# revision 19
# baseline (speedup 1.0000x reference)
"""Grouped-query attention (B=2,T=2048,D=2048, 4 groups x 4 heads x 128d) on 8 trn2 cores.

Sharding: core = (batch b, group g); b = core//4, g = core%4. Each core computes its
group's QKV projections, QK-rmsnorm+rope, causal attention, and a partial output
projection o_g @ wo_g; the host sums the 4 per-group partials per batch.

v2 design (vs the phase-split baseline):
  * Single fused loop over 512-wide query chunks tf=0..3: projections -> attention ->
    output projection per chunk, so DMA, Act and PE overlap across stages and output
    DMA spreads over the whole kernel.
  * Softmax denominators accumulated on the Vector engine (tensor adds over exp
    tiles) + one tiny f32r ones-matmul per head, instead of 160 ones-matmul chains
    on the PE.
  * K rmsnorm folded into the exp scale (per-partition scalar, no broadcast); q
    rmsnorm+gain fused into one scalar_tensor_tensor op with a single batched
    rsqrt-broadcast round trip per chunk.
  * Causal mask added via identity-matmul accumulation into the scores PSUM.
  * Heads paired in a [128,1024] PSUM tile so exp runs half as many, twice as wide.
  * Output-projection matmuls queued as "fillers" and interleaved into the (Act-
    bound) attention inner loop to keep the PE busy; output written as bf16 and
    summed on host.
  * Bulk loads consolidated into single multi-dim DMAs so the SP queue reaches the
    per-chunk round trips quickly; per-chunk x loads issued lazily.
"""

import sys
from contextlib import ExitStack

for _p in ("/opt/trn_rl_repo", "/opt/pypackages"):
    if _p not in sys.path:
        sys.path.insert(0, _p)

import numpy as np
import ml_dtypes

import concourse.bass as bass
import concourse.mybir as mybir
import concourse.tile as tile
from concourse import bacc
from concourse.bass_utils import run_bass_kernel_spmd

bf16 = ml_dtypes.bfloat16
BF = mybir.dt.bfloat16
F32 = mybir.dt.float32
F32R = mybir.dt.float32r
AF = mybir.ActivationFunctionType
MUL = mybir.AluOpType.mult

B, T, D = 2, 2048, 2048
HD, H, G = 128, 4, 4
KC = D // 128           # 16 contraction chunks
NTF = T // 512          # 4 query chunks
EPS = 1e-6
MULT2 = float(HD) ** -0.5   # mult^2 folded into q gains

_NC_CACHE = {}


def _bcast(ap, p=128):
    """Partition-broadcast AP: [1, N] row -> [p, N] (step-0 partition dim)."""
    return bass.AP(tensor=ap.tensor, offset=ap.offset, ap=[[0, p]] + [list(a) for a in ap.ap[1:]])


def _colview(ap, n):
    """DRAM [1, n*128] row -> [128, n] column view (partition stride 1 elem)."""
    return bass.AP(tensor=ap.tensor, offset=ap.offset, ap=[[1, 128], [128, n]])


def _pairv(t, off):
    """[128,1024] tile -> 2-level AP over cols [off:512] u [512+off:1024]."""
    if off == 0:
        return t[:, :]
    return bass.AP(tensor=t.tensor, offset=t.offset + off,
                   ap=[list(t.ap[0]), [512, 2], [1, 512 - off]])


def _blk3(t, nmid, w):
    """SBUF tile [128, nmid*w] -> 3D AP [128, nmid, w] (contiguous)."""
    return bass.AP(tensor=t.tensor, offset=t.offset,
                   ap=[list(t.ap[0]), [w, nmid], [1, w]])


def _dram3(d, offset, pstride, midstride, nmid, w):
    """DRAM 3D gather AP: [128 partitions x pstride, nmid x midstride, w]."""
    ap = d[:, :]
    return bass.AP(tensor=ap.tensor, offset=ap.offset + offset,
                   ap=[[pstride, 128], [midstride, nmid], [1, w]])


def _build_nc():
    nc = bacc.Bacc(None)

    xt_d = nc.declare_dram_parameter("xt", [D, T], BF, isOutput=False)
    wq_d = nc.declare_dram_parameter("wq", [D, H * HD], BF, isOutput=False)
    wk_d = nc.declare_dram_parameter("wk", [D, HD], BF, isOutput=False)
    wv_d = nc.declare_dram_parameter("wv", [D, HD], BF, isOutput=False)
    wo_d = nc.declare_dram_parameter("wo", [H * HD, D], BF, isOutput=False)
    gqs_d = nc.declare_dram_parameter("gqs", [HD, H], F32, isOutput=False)
    gks_d = nc.declare_dram_parameter("gks", [HD, 1], F32, isOutput=False)
    cos_d = nc.declare_dram_parameter("cosf", [HD, T], BF, isOutput=False)
    sin_d = nc.declare_dram_parameter("sins", [HD, T], BF, isOutput=False)
    msk_d = nc.declare_dram_parameter("mask", [128, 128], BF, isOutput=False)
    idn_d = nc.declare_dram_parameter("ident", [128, 128], BF, isOutput=False)
    out_d = nc.declare_dram_parameter("out", [T, D], BF, isOutput=True)

    with tile.TileContext(nc) as tc:
        with ExitStack() as S:
            # PSUM: sc pairs 2x4KB + po 2x2KB + row 1x2KB + fil 1x2KB = 16KB
            sc_p = S.enter_context(tc.tile_pool(name="sc", bufs=2, space="PSUM"))
            po_p = S.enter_context(tc.tile_pool(name="po", bufs=2, space="PSUM"))
            row_p = S.enter_context(tc.tile_pool(name="row", bufs=1, space="PSUM"))
            fil_p = S.enter_context(tc.tile_pool(name="fil", bufs=1, space="PSUM"))
            pers = S.enter_context(tc.tile_pool(name="pers", bufs=1))
            ring = S.enter_context(tc.tile_pool(name="ring", bufs=2))
            dram_p = S.enter_context(tc.tile_pool(name="dramb", bufs=2, space="DRAM"))

            # ---------------- persistent tiles ----------------
            gqs = pers.tile([HD, H], F32, tag="gqs", name="gqs")
            gks = pers.tile([HD, 1], F32, tag="gks", name="gks")
            maskT = pers.tile([128, 128], BF, tag="mask", name="maskT")
            ident = pers.tile([128, 128], BF, tag="ident", name="ident")
            ones_bf = pers.tile([128, 1], BF, tag="ones", name="ones")
            eps_t = pers.tile([1, 1], F32, tag="eps", name="eps")
            khat = pers.tile([128, T], BF, tag="khat", name="khat")
            vnats = [pers.tile([128, 512], BF, tag=f"vn{tf}", name=f"vn{tf}")
                     for tf in range(NTF)]
            accp = [pers.tile([128, 1024], BF, tag=f"acc{p}", name=f"acc{p}")
                    for p in range(2)]
            xts = [pers.tile([128, KC * 512], BF, tag=f"xt{tf}", name=f"xt{tf}")
                   for tf in range(NTF)]
            wq_b = pers.tile([128, KC * 512], BF, tag="wq", name="wq")
            wk_b = pers.tile([128, KC * 128], BF, tag="wk", name="wk")
            wv_b = pers.tile([128, KC * 128], BF, tag="wv", name="wv")
            wo_b = pers.tile([128, H * D], BF, tag="wo", name="wo")
            cosf = pers.tile([HD, T], BF, tag="cosf", name="cosf")
            sins = pers.tile([HD, T], BF, tag="sins", name="sins")

            nc.vector.memset(ones_bf, 1.0)
            nc.vector.memset(eps_t, EPS)

            # ---------------- upfront DMAs (consolidated; SP order = priority) --
            for q0 in range(0, KC, 4):
                nc.sync.dma_start(
                    out=bass.AP(tensor=wq_b.tensor, offset=wq_b.offset + q0 * 512,
                                ap=[list(wq_b.ap[0]), [512, 4], [1, 512]]),
                    in_=_dram3(wq_d, q0 * 128 * 512, 512, 128 * 512, 4, 512))
                nc.sync.dma_start(
                    out=bass.AP(tensor=xts[0].tensor, offset=xts[0].offset + q0 * 512,
                                ap=[list(xts[0].ap[0]), [512, 4], [1, 512]]),
                    in_=_dram3(xt_d, q0 * 128 * T, T, 128 * T, 4, 512))
            nc.sync.dma_start(out=gqs, in_=gqs_d[:, :])
            nc.sync.dma_start(out=gks, in_=gks_d[:, :])
            nc.sync.dma_start(out=maskT, in_=msk_d[:, :])
            nc.sync.dma_start(out=ident, in_=idn_d[:, :])
            nc.sync.dma_start(out=_blk3(wk_b, KC, 128),
                              in_=_dram3(wk_d, 0, 128, 128 * 128, KC, 128))
            nc.sync.dma_start(out=_blk3(wv_b, KC, 128),
                              in_=_dram3(wv_d, 0, 128, 128 * 128, KC, 128))
            nc.sync.dma_start(out=cosf, in_=cos_d[:, :])
            nc.sync.dma_start(out=sins, in_=sin_d[:, :])

            def xchunk(tf, kc):
                return xts[tf][:, kc * 512:(kc + 1) * 512]

            def wq_l(kc, h):
                return wq_b[:, kc * 512 + h * 128:kc * 512 + (h + 1) * 128]

            def wo_l(h, oc):
                return wo_b[:, h * D + oc * 512:h * D + (oc + 1) * 512]

            # ---------------- filler queue (deferred output projection) ------
            filler_q = []
            _alt = [0]
            _ost_cur = {}

            def _fil_ps():
                return fil_p.tile([128, 512], F32, tag="fil", name="fil")

            def emit_filler(n=1):
                for _ in range(n):
                    if not filler_q:
                        return
                    filler_q.pop(0)(_fil_ps)

            def queue_outproj(tf, oTs):
                for tb4 in range(4):
                    for oc in range(4):
                        def unit(get_ps, tail=False, tf=tf, tb4=tb4, oc=oc, oTs=oTs):
                            pso = get_ps()
                            for h in range(H):
                                nc.tensor.matmul(pso, oTs[h][:, tb4 * 128:(tb4 + 1) * 128],
                                                 wo_l(h, oc), start=(h == 0), stop=(h == H - 1))
                            tb = 4 * tf + tb4
                            _alt[0] ^= 1
                            if tail:
                                ost = ring.tile([128, 512], BF, tag="ostt", name="ostt", bufs=3)
                                if _alt[0]:
                                    nc.vector.tensor_copy(out=ost, in_=pso)
                                else:
                                    nc.scalar.copy(out=ost, in_=pso)
                                q = nc.sync if oc % 2 else nc.scalar
                                q.dma_start(out=out_d[tb * 128:(tb + 1) * 128,
                                                      oc * 512:(oc + 1) * 512], in_=ost)
                            else:
                                if oc == 0:
                                    _ost_cur[0] = ring.tile([128, 2048], BF, tag="ost",
                                                            name="ost", bufs=2)
                                ost = _ost_cur[0]
                                if _alt[0]:
                                    nc.vector.tensor_copy(out=ost[:, oc * 512:(oc + 1) * 512],
                                                          in_=pso)
                                else:
                                    nc.scalar.copy(out=ost[:, oc * 512:(oc + 1) * 512],
                                                   in_=pso)
                                if oc == 3:
                                    nc.scalar.dma_start(out=out_d[tb * 128:(tb + 1) * 128, :],
                                                        in_=ost)
                        filler_q.append(unit)

            # ---------------- fused per-chunk pipeline ----------------
            # Emission order: Ape(0), Anorm(0), then per tf: Ape(tf+1) [PE work
            # that covers the Anorm round-trip latency], B(tf) [attention +
            # interleaved output-projection fillers of tf-1], Anorm(tf+1).
            def rowsum(rowq, ps_half, dst_off):
                sq = ring.tile([128, 512], BF, tag="sq", name="sq", bufs=2)
                nc.scalar.square(out=sq, in_=ps_half)
                pr = row_p.tile([1, 512], F32, tag="row", name="pr")
                nc.tensor.matmul(pr, ones_bf, sq, start=True, stop=True)
                nc.vector.tensor_copy(out=rowq[:, dst_off:dst_off + 512], in_=pr)

            def rope(tf, src, dest):
                sl = slice(tf * 512, (tf + 1) * 512)
                sw = ring.tile([128, 512], BF, tag="sw", name="sw", bufs=2)
                nc.sync.dma_start(out=sw[0:64, :], in_=src[64:128, :])
                nc.sync.dma_start(out=sw[64:128, :], in_=src[0:64, :])
                t1 = ring.tile([128, 512], BF, tag="t1", name="t1", bufs=2)
                nc.vector.tensor_mul(t1, src, cosf[:, sl])
                nc.gpsimd.tensor_mul(sw, sw, sins[:, sl])
                nc.vector.tensor_add(dest, t1, sw)

            def stage_a_pe(tf):
                """Projections + rowsums + V transpose; all PE-driven work."""
                if tf == 0:        # bulk loads on the Act DGE ring, issued
                    nc.scalar.dma_start(out=_blk3(wo_b, H, D),   # after startup
                                        in_=_dram3(wo_d, 0, D, 128 * D, H, D))
                if tf + 1 < NTF:   # lazy prefetch of next chunk's x
                    nc.scalar.dma_start(out=_blk3(xts[tf + 1], KC, 512),
                                        in_=_dram3(xt_d, (tf + 1) * 512, T, 128 * T, KC, 512))
                st = {"rowq": ring.tile([1, 2560], F32, tag="rowq", name="rowq", bufs=1),
                      "qraw": []}

                def qside(pair, half, h):
                    ph = pair[:, half * 512:half * 512 + 512]
                    qr = ring.tile([128, 512], BF, tag="qraw", name=f"qraw{h}", bufs=8)
                    nc.scalar.copy(out=qr, in_=ph)
                    st["qraw"].append(qr)
                    rowsum(st["rowq"], ph, 512 + h * 512)

                pair01 = sc_p.tile([128, 1024], F32, tag="sc", name="pq01")
                for kc in range(KC):
                    for half in range(2):
                        nc.tensor.matmul(pair01[:, half * 512:half * 512 + 512],
                                         wq_l(kc, half), xchunk(tf, kc),
                                         start=(kc == 0), stop=(kc == KC - 1))
                pair23 = sc_p.tile([128, 1024], F32, tag="sc", name="pq23")
                for kc in range(KC):
                    for half in range(2):
                        nc.tensor.matmul(pair23[:, half * 512:half * 512 + 512],
                                         wq_l(kc, 2 + half), xchunk(tf, kc),
                                         start=(kc == 0), stop=(kc == KC - 1))
                qside(pair01, 0, 0)
                qside(pair01, 1, 1)
                pairkv = sc_p.tile([128, 1024], F32, tag="sc", name="pkv")
                for kc in range(KC):
                    nc.tensor.matmul(pairkv[:, 0:512],
                                     wk_b[:, kc * 128:(kc + 1) * 128], xchunk(tf, kc),
                                     start=(kc == 0), stop=(kc == KC - 1))
                    nc.tensor.matmul(pairkv[:, 512:1024],
                                     wv_b[:, kc * 128:(kc + 1) * 128], xchunk(tf, kc),
                                     start=(kc == 0), stop=(kc == KC - 1))
                qside(pair23, 0, 2)
                qside(pair23, 1, 3)
                rowsum(st["rowq"], pairkv[:, 0:512], 0)
                kn = ring.tile([128, 512], BF, tag="qn", name="kn", bufs=3)
                nc.vector.tensor_scalar_mul(kn, pairkv[:, 0:512], gks[:, 0:1])
                st["kn"] = kn
                vtr = ring.tile([128, 512], BF, tag="vtr", name="vtr", bufs=1)
                nc.scalar.copy(out=vtr, in_=pairkv[:, 512:1024])
                ftp = _fil_ps().bitcast(BF)
                for tb4 in range(4):
                    nc.tensor.transpose(ftp[:, tb4 * 128:(tb4 + 1) * 128],
                                        vtr[:, tb4 * 128:(tb4 + 1) * 128], ident)
                nc.vector.tensor_copy(out=vnats[tf], in_=ftp[:, 0:512])
                return st

            def stage_a_norm(tf, st):
                """rsqrt round trip, qk normalization, rope (no PE work)."""
                rowq = st["rowq"]
                nc.scalar.activation(out=rowq, in_=rowq, func=AF.Sqrt,
                                     bias=eps_t[:, 0:1], scale=1.0 / HD)
                nc.vector.reciprocal_approx_fast(out=rowq, in_=rowq)
                rqd = dram_p.tile([1, 2560], F32, tag="rqd", name="rqd")
                nc.sync.dma_start(out=rqd, in_=rowq)
                rb5 = ring.tile([128, 2560], F32, tag="rb5", name="rb5", bufs=1)
                nc.sync.dma_start(out=rb5, in_=_bcast(rqd))
                t2 = ring.tile([128, 512], BF, tag="t2", name="t2", bufs=1)
                rope(tf, st["kn"], t2)
                nc.vector.tensor_mul(khat[:, tf * 512:(tf + 1) * 512], t2, rb5[:, 0:512])
                qhat = []
                for h in range(H):
                    qn = ring.tile([128, 512], BF, tag="qn", name=f"qn{h}", bufs=3)
                    nc.vector.scalar_tensor_tensor(out=qn, in0=st["qraw"][h],
                                                   scalar=gqs[:, h:h + 1],
                                                   in1=rb5[:, 512 + h * 512:512 + (h + 1) * 512],
                                                   op0=MUL, op1=MUL)
                    qh = ring.tile([128, 512], BF, tag=f"qh{h}", name=f"qh{h}", bufs=2)
                    rope(tf, qn, qh)
                    qhat.append(qh)
                return qhat

            def stage_b(tf, qhat, mid=None):
                """Causal attention for chunk tf, heads in pairs; queues outproj."""
                njb = 4 * tf + 4
                oTs = []
                for pi, (ha, hb) in enumerate(((0, 1), (2, 3))):
                    pos = [po_p.tile([128, 512], F32, tag="po", name=f"po{h}")
                           for h in (ha, hb)]

                    def pv(prev):
                        pjb, poff, ppt = prev
                        vb = vnats[pjb // 4][:, (pjb % 4) * 128:(pjb % 4) * 128 + 128]
                        for half in range(2):
                            nc.tensor.matmul(pos[half][:, poff:], vb,
                                             ppt[:, half * 512 + poff:(half + 1) * 512],
                                             start=(pjb == 0), stop=(pjb == njb - 1))

                    prev = None
                    for jb in range(njb):
                        off = max(0, 128 * (jb - 4 * tf))
                        diag = jb >= 4 * tf
                        kb = khat[:, jb * 128:(jb + 1) * 128]
                        ps = sc_p.tile([128, 1024], F32, tag="sc", name="ps")
                        for half, h in enumerate((ha, hb)):
                            base = half * 512
                            if diag:
                                nc.tensor.matmul(ps[:, base + off:base + off + 128], kb,
                                                 qhat[h][:, off:off + 128], start=True, stop=False)
                                nc.tensor.matmul(ps[:, base + off:base + off + 128], ident,
                                                 maskT, start=False, stop=True)
                                if off < 384:
                                    nc.tensor.matmul(ps[:, base + off + 128:base + 512], kb,
                                                     qhat[h][:, off + 128:512],
                                                     start=True, stop=True)
                            else:
                                nc.tensor.matmul(ps[:, base:base + 512], kb, qhat[h],
                                                 start=True, stop=True)
                        pt = ring.tile([128, 1024], BF, tag="pt", name="pt", bufs=4)
                        nc.scalar.activation(out=_pairv(pt, off), in_=_pairv(ps, off),
                                             func=AF.Exp)
                        if jb == 0:
                            nc.vector.tensor_copy(out=accp[pi], in_=pt)
                        else:
                            nc.vector.tensor_add(_pairv(accp[pi], off), _pairv(accp[pi], off),
                                                 _pairv(pt, off))
                        if prev is not None:
                            pv(prev)
                        if jb % 2 == 1:
                            emit_filler(1)
                        prev = (jb, off, pt)
                    pv(prev)
                    # denominators -> reciprocal -> broadcast -> oT
                    dnrow = ring.tile([1, 1024], F32, tag="dnrow", name="dnrow", bufs=1)
                    for half in range(2):
                        pr = row_p.tile([1, 512], F32, tag="row", name="pdn")
                        nc.tensor.matmul(pr, ones_bf,
                                         accp[pi][:, half * 512:half * 512 + 512],
                                         start=True, stop=True)
                        nc.vector.tensor_copy(out=dnrow[:, half * 512:half * 512 + 512], in_=pr)
                    nc.vector.reciprocal_approx_fast(out=dnrow, in_=dnrow)
                    dnd = dram_p.tile([1, 1024], F32, tag="dnd", name="dnd")
                    nc.sync.dma_start(out=dnd, in_=dnrow)
                    db = ring.tile([128, 1024], F32, tag="db", name="db", bufs=1)
                    nc.sync.dma_start(out=db, in_=_bcast(dnd))
                    for half, h in enumerate((ha, hb)):
                        oT = ring.tile([128, 512], BF, tag=f"oT{h}", name=f"oT{h}", bufs=2)
                        nc.vector.tensor_mul(oT, pos[half], db[:, half * 512:half * 512 + 512])
                        oTs.append(oT)
                    emit_filler(3 if pi == 0 else 2)
                    if pi == 0 and mid is not None:
                        mid()
                queue_outproj(tf, oTs)

            st = stage_a_pe(0)
            qh_cur = stage_a_norm(0, st)
            for tf in range(NTF):
                cell = {}
                mid = None
                if tf + 1 < NTF:
                    st_next = stage_a_pe(tf + 1)

                    def mid(c=cell, t=tf + 1, s=st_next):
                        c["q"] = stage_a_norm(t, s)
                stage_b(tf, qh_cur, mid)
                qh_cur = cell.get("q")

            # tail: drain remaining output-projection units (3 psum slots)
            while filler_q:
                pair = sc_p.tile([128, 1024], F32, tag="sc", name="tail")
                filler_q.pop(0)(lambda: pair[:, 0:512], tail=True)
                if filler_q:
                    filler_q.pop(0)(lambda: pair[:, 512:1024], tail=True)
                if filler_q:
                    filler_q.pop(0)(_fil_ps, tail=True)
    nc.finalize()
    return nc


def _rope_tables():
    d = np.arange(64, dtype=np.float64)
    ang = 10000.0 ** (-d / 64.0)
    pos = np.arange(T, dtype=np.float64)
    rad = pos[None, :] * ang[:, None]          # [64, T]
    cos, sin = np.cos(rad), np.sin(rad)
    cosF = np.concatenate([cos, cos], 0).astype(bf16)
    sinS = np.concatenate([-sin, sin], 0).astype(bf16)
    return np.ascontiguousarray(cosF), np.ascontiguousarray(sinS)


def _in_maps(x, wq, wk, wv, wo, gq, gk):
    cosF, sinS = _rope_tables()
    mask = np.ascontiguousarray(np.triu(np.full((128, 128), -1e9, np.float32), 1).T.astype(bf16))
    ident = np.eye(128, dtype=bf16)
    maps = []
    for core in range(8):
        b, g = core // 4, core % 4
        maps.append({
            "xt": np.ascontiguousarray(x[b].T).astype(bf16),
            "wq": np.ascontiguousarray(wq[:, g * 512:(g + 1) * 512]).astype(bf16),
            "wk": np.ascontiguousarray(wk[:, g * 128:(g + 1) * 128]).astype(bf16),
            "wv": np.ascontiguousarray(wv[:, g * 128:(g + 1) * 128]).astype(bf16),
            "wo": np.ascontiguousarray(wo[g * 512:(g + 1) * 512, :]).astype(bf16),
            "gqs": np.ascontiguousarray((gq[g].T * MULT2).astype(np.float32)),
            "gks": np.ascontiguousarray(gk[g].astype(np.float32).reshape(HD, 1)),
            "cosf": cosF, "sins": sinS, "mask": mask, "ident": ident,
        })
    return maps


def _get_nc():
    if "nc" not in _NC_CACHE:
        _NC_CACHE["nc"] = _build_nc()
    return _NC_CACHE["nc"]


def _run(inputs, trace=False, trace_kwargs=None, tmpdir=None):
    nc = _get_nc()
    maps = _in_maps(inputs["x"], inputs["wq"], inputs["wk"], inputs["wv"],
                    inputs["wo"], inputs["gq"], inputs["gk"])
    res = run_bass_kernel_spmd(nc, maps, core_ids=list(range(8)), trace=trace,
                               tmpdir=tmpdir, **(trace_kwargs or {}))
    out = np.zeros((B, T, D), np.float32)
    for core in range(8):
        out[core // 4] += res.results[core]["out"].astype(np.float32)
    return out, res


def kernel(**inputs):
    inputs = {k: np.asarray(v) for k, v in inputs.items()}
    out, _ = _run(inputs, trace=False)
    return out
